# revision 63
# baseline (speedup 1.0000x reference)
"""Bass/Tile multi-head-attention kernel for Trainium2, SPMD over 8 NeuronCores.

Sharding: core c = bs*2 + qhalf  (batch-parallel x query-half).  Each core
computes the full output rows for its (batch, 1024-query) slice; host glue
only slices / transposes / concatenates (no arithmetic on host).

Device math per core (bs, q0):
  QpT = (WQ^T)^T-contract (qm . Q)^T        [d, q]   (mask folded into PSUM evac)
  KpT = ... (km . K)^T                      [d, k]
  Vp  = (km . V) proj, heads interleaved    [k, 8, 65] with ones col at 64
  per head h, q-block qb (512):
    S^T[k, q] = KpT_h^T-slice . QpT_h       (PE, contraction d=64)
    E = exp(S^T / 8)                        (ACT, PSUM->SBUF)
    EP = E * mask^T                         (DVE, bf16 2x)
    [Y^T; rowsum] += [Vp_h | 1]^T . EP      (PE, accumulated over k)
    Y^T *= km(q) / rowsum                   (recip + partition_broadcast + DVE)
  out^T = WO^T-contract . Y^T               (PE)  -> DRAM [e, q] fp32
"""

import numpy as np
import ml_dtypes

import concourse.bass as bass
import concourse.bacc as bacc
import concourse.mybir as mybir
import concourse.tile as tile
from concourse import bass_utils

BS, N, D, H, DK = 4, 2048, 512, 8, 64
NCORES = 8
NQ = N // 2          # queries per core
QB = 512             # query block
KC = N // 128        # 16 key chunks of 128
F32 = mybir.dt.float32
BF16 = mybir.dt.bfloat16

# compute dtype for matmuls / staged activations: "bf16" or "f32"
COMPUTE = "bf16"
CDT = BF16 if COMPUTE == "bf16" else F32
NP_CDT = ml_dtypes.bfloat16 if COMPUTE == "bf16" else np.float32

# debug/bisection knobs (all default to production behavior)
FLAGS = {
    "bcast": "gpsimd",   # "gpsimd" | "dram"  — scale row broadcast mechanism
    "const_scale": False,  # replace recip+kmq by 1.0 (timing bisect only)
    "no_av": False,       # skip AV matmuls + evac (timing bisect only)
    "av_h0": False,       # all AV lhsT from head-0 slice (alignment probe)
    "no_exp": False,      # skip exp+mask (AV consumes garbage; timing only)
    "no_attn": False,     # skip whole attention loop (timing only)
    "reps": 1,            # emit the body N times (device-time measurement)
    "lookahead": 9,       # AV pipeline lookahead (groups)
    "proj_in_pss": False,  # proj psum shares the S-tile pool (frees a bank for psy)
    "psy_bufs": 3,
    "et_bufs": 6,
    "ep_bufs": 9,
    "gps_mask_every": 0,  # every Nth mask-mul runs on GPSIMD (0 = never)
    "warmup_mms": 20,     # dummy matmuls during the DMA ramp (HAM warm-up) +
                          # a dummy Exp to preload the ACT table set
}


def _emit(nc, t):
    """Emit the whole per-core program inside a TileContext."""
    with tile.TileContext(nc) as tc:
        for rep in range(int(FLAGS["reps"])):
            _emit_body(nc, tc, t, sfx=f"r{rep}" if rep else "")


def _emit_body(nc, tc, t, sfx=""):
    import contextlib
    ctx = contextlib.ExitStack()
    with ctx:
        persist = ctx.enter_context(tc.tile_pool(name="persist" + sfx, bufs=1))
        raw = ctx.enter_context(tc.tile_pool(name="raw" + sfx, bufs=1))

        # ---- small constants (loads emitted after the critical kt wave) ----
        qm_b = persist.tile([128, NQ], BF16, tag="qm_b")
        km_b = persist.tile([128, N], BF16, tag="km_b")
        km_sbr = persist.tile([128, KC], BF16, tag="km_sbr")
        km_sb = persist.tile([128, KC], F32, tag="km_sb")
        kmq_sb = persist.tile([1, NQ], BF16, tag="kmq_sb")

        def load_consts():
            ap = t["qmr"].ap()
            nc.sync.dma_start(out=qm_b[:], in_=bass.AP(tensor=ap.tensor, offset=ap.offset,
                                                       ap=[[0, 128], [1, NQ]]))
            ap = t["kmr"].ap()
            nc.sync.dma_start(out=km_b[:], in_=bass.AP(tensor=ap.tensor, offset=ap.offset,
                                                       ap=[[0, 128], [1, N]]))
            nc.sync.dma_start(out=km_sbr[:], in_=bass.AP(tensor=ap.tensor, offset=ap.offset,
                                                         ap=[[1, 128], [128, KC]]))
            nc.vector.tensor_copy(out=km_sb[:], in_=km_sbr[:])
            nc.sync.dma_start(out=kmq_sb[:], in_=t["kmq"].ap())

        # ---- weights (tiles created; loads emitted in criticality order) ----
        w_sb = {}
        for wname in ("wqt", "wkt", "wvt", "wot"):
            w_sb[wname] = [persist.tile([128, D], CDT, tag=f"{wname}{ct}", name=f"{wname}{ct}")
                           for ct in range(4)]

        def load_w(wname):
            for ct in range(4):
                nc.gpsimd.dma_start(out=w_sb[wname][ct][:],
                                    in_=t[wname].ap()[ct * 128:(ct + 1) * 128, :])

        # ---- persistent result tensors -------------------------------------
        qpt_sb = [persist.tile([128, NQ], CDT, tag=f"qpt{i}", name=f"qpt{i}") for i in range(4)]
        kpt_sb = [persist.tile([128, N], CDT, tag=f"kpt{i}", name=f"kpt{i}") for i in range(4)]
        v_sb = [persist.tile([128, H, DK + 1], CDT, tag=f"v{i}", name=f"v{i}") for i in range(KC)]
        yt_sb = [persist.tile([128, NQ], CDT, tag=f"yt{i}", name=f"yt{i}") for i in range(4)]
        if FLAGS["no_attn"] or FLAGS["no_av"]:
            for yt in yt_sb:
                nc.vector.memset(yt[:], 0.0)
        mt_ap = t["mt"].ap()

        # raw activations (alive for the whole kernel; proj is dc-interleaved)
        # loads split into column halves and emitted in criticality order:
        # qt+wqt (first proj) -> kt+wkt (first S-MM) -> vt+wvt -> wot
        qt_sb, kt_sb, vt_sb = [], [], []
        for nm, lst, cols in (("qt", qt_sb, NQ), ("kt", kt_sb, N), ("vt", vt_sb, N)):
            for ct in range(4):
                lst.append(raw.tile([128, cols], CDT, tag=f"{nm}{ct}", name=f"{nm}{ct}"))

        def load_raw(nm, lst, cols):
            for ct in range(4):
                for hlf in range(2):
                    c0 = hlf * (cols // 2)
                    nc.sync.dma_start(out=lst[ct][:, c0:c0 + cols // 2],
                                      in_=t[nm].ap()[ct * 128:(ct + 1) * 128, c0:c0 + cols // 2])

        mpool = ctx.enter_context(tc.tile_pool(name="mts" + sfx, bufs=1))
        mts = {}

        def load_mt(kc, qb):
            mtile = mpool.tile([128, QB], BF16, tag=f"mt{kc}_{qb}", name=f"mt{kc}_{qb}")
            off = (kc * 128) * NQ + qb * QB
            eng = nc.gpsimd if (kc + qb) % 2 else nc.sync
            eng.dma_start(out=mtile[:],
                          in_=bass.AP(tensor=mt_ap.tensor, offset=mt_ap.offset + off,
                                      ap=[[NQ, 128], [1, QB]]))
            mts[(kc, qb)] = mtile

        load_consts()
        load_raw("qt", qt_sb, NQ)
        load_w("wqt")
        load_raw("kt", kt_sb, N)
        load_w("wkt")
        load_raw("vt", vt_sb, N)
        load_w("wvt")
        for kc in range(4):
            load_mt(kc, 0)
        load_w("wot")

        pp = None if FLAGS["proj_in_pss"] else ctx.enter_context(tc.tile_pool(name="psproj" + sfx, bufs=1, space="PSUM"))

        def proj_ps():
            if FLAGS["proj_in_pss"]:
                return pool_s.tile([128, 2 * QB], F32, tag="pss", name="ps")[:, 0:QB]
            return pp.tile([128, QB], F32, tag="ps", name="ps")

        def emit_proj_q_blk(dc, blk):
            ps = proj_ps()
            for ct in range(4):
                nc.tensor.matmul(ps[:], w_sb["wqt"][ct][:, dc * 128:(dc + 1) * 128],
                                 qt_sb[ct][:, blk * QB:(blk + 1) * QB],
                                 start=(ct == 0), stop=(ct == 3))
            nc.vector.tensor_mul(qpt_sb[dc][:, blk * QB:(blk + 1) * QB],
                                 ps[:], qm_b[:, blk * QB:(blk + 1) * QB])

        def emit_proj_k_blk(dc, blk):
            ps = proj_ps()
            for ct in range(4):
                nc.tensor.matmul(ps[:], w_sb["wkt"][ct][:, dc * 128:(dc + 1) * 128],
                                 kt_sb[ct][:, blk * QB:(blk + 1) * QB],
                                 start=(ct == 0), stop=(ct == 3))
            nc.vector.tensor_mul(kpt_sb[dc][:, blk * QB:(blk + 1) * QB],
                                 ps[:], km_b[:, blk * QB:(blk + 1) * QB])

        def emit_proj_qk(dc):
            for blk in range(NQ // QB):
                emit_proj_q_blk(dc, blk)
            for blk in range(N // QB):
                emit_proj_k_blk(dc, blk)

        def emit_proj_v(kc):
            ps = proj_ps()
            for ct in range(4):
                nc.tensor.matmul(ps[:], vt_sb[ct][:, kc * 128:(kc + 1) * 128],
                                 w_sb["wvt"][ct][:], start=(ct == 0), stop=(ct == 3))
            nc.vector.tensor_scalar_mul(v_sb[kc][:, :, 0:DK],
                                        ps.rearrange("p (h e) -> p h e", h=H),
                                        km_sb[:, kc:kc + 1])
            nc.gpsimd.memset(v_sb[kc][:, :, DK:DK + 1], 1.0)

        # ---- attention (dc-interleaved with projections) --------------------
        pool_s = ctx.enter_context(tc.tile_pool(name="pss" + sfx, bufs=2, space="PSUM"))
        pool_y = ctx.enter_context(tc.tile_pool(name="psy" + sfx, bufs=2, space="PSUM"))
        epool = ctx.enter_context(tc.tile_pool(name="eps" + sfx, bufs=1))
        spool = ctx.enter_context(tc.tile_pool(name="smalls" + sfx, bufs=4))
        opool = ctx.enter_context(tc.tile_pool(name="osb" + sfx, bufs=3))

        psy = {}
        pending = []

        # PE/ACT warm-up during the input-DMA ramp: dummy matmuls (no DMA
        # deps) release the HAM clock gate, and a dummy Exp pulls the ACT
        # table load off the first real exp's critical path.
        nwarm = int(FLAGS["warmup_mms"])
        if nwarm:
            # borrow kpt_sb[0] pre-write (WAR: its proj evac waits on these)
            # and dump into yt_sb[0], which is fully overwritten by yn evacs.
            scratch = kpt_sb[0]
            nc.vector.memset(scratch[:, 0:QB], 0.0)
            ps_w = pp.tile([128, QB], F32, tag="ps", name="ps_w") if pp is not None \
                else pool_s.tile([128, 2 * QB], F32, tag="pss", name="ps_w")[:, 0:QB]
            for _ in range(nwarm):
                nc.tensor.matmul(ps_w[:], scratch[:, 0:128], scratch[:, 0:QB],
                                 start=True, stop=True)
            nc.scalar.activation(out=yt_sb[0][:, 0:QB], in_=ps_w[:],
                                 func=mybir.ActivationFunctionType.Exp, scale=0.125)


        if not FLAGS["no_attn"]:
            # Head-pair stream: per (dc, qb, kc) one psum tile holds
            # [S^T(h0,kc) | S^T(h1,kc)] (partition-offset 0/64 lhsT slices run
            # concurrently on disjoint PE row groups). One exp over both, two
            # mask muls sharing one mask tile, two AV accumulations.
            def V(kc):
                return lambda: emit_proj_v(kc)

            def Kb(dc, blk):
                return lambda: emit_proj_k_blk(dc, blk)

            def Qb(dc, blk):
                return lambda: emit_proj_q_blk(dc, blk)

            bg_by_dc = {
                0: [V(kc) for kc in range(6, KC)]
                   + [Qb(1, 0), Qb(1, 1), Kb(1, 0), Kb(1, 1), Kb(1, 2), Kb(1, 3)],
                1: [Qb(2, 0), Qb(2, 1), Kb(2, 0), Kb(2, 1), Kb(2, 2), Kb(2, 3)],
                2: [Qb(3, 0), Qb(3, 1), Kb(3, 0), Kb(3, 1), Kb(3, 2), Kb(3, 3)],
            }
            emit_proj_qk(0)
            for kc in range(6):
                emit_proj_v(kc)

            def emit_av(qb, h0, kc, ep):
                for half, h in ((0, h0), (1, h0 + 1)):
                    hh = 0 if FLAGS["av_h0"] else h
                    nc.tensor.matmul(psy[(h, qb)][0:DK + 1, :], v_sb[kc][:, hh, :],
                                     ep[:, half * QB:(half + 1) * QB],
                                     start=(kc == 0), stop=(kc == KC - 1))
                if kc == KC - 1:
                    for h in (h0, h0 + 1):
                        ps_y = psy[(h, qb)]
                        scl = spool.tile([1, QB], F32, tag="scl", name="scl")
                        if FLAGS["const_scale"]:
                            nc.vector.memset(scl[:], 1.0)
                        else:
                            rec = spool.tile([1, QB], F32, tag="rec", name="rec")
                            nc.vector.reciprocal(rec[:], ps_y[DK:DK + 1, :])
                            nc.vector.tensor_mul(scl[:], rec[:],
                                                 kmq_sb[:, qb * QB:(qb + 1) * QB])
                        sclb = spool.tile([DK, QB], F32, tag="sclb", name="sclb")
                        nc.gpsimd.partition_broadcast(sclb[:], scl[:])
                        po = (h % 2) * DK
                        nc.vector.tensor_mul(yt_sb[h // 2][po:po + DK,
                                                           qb * QB:(qb + 1) * QB],
                                             ps_y[0:DK, :], sclb[:])

            for dc in range(4):
                h0 = 2 * dc
                bg = bg_by_dc.get(dc, [])
                for qb in range(NQ // QB):
                    psy[(h0, qb)] = pool_y.tile([128, QB], F32, tag="psy",
                                                name="psy", bufs=int(FLAGS["psy_bufs"]))
                    psy[(h0 + 1, qb)] = pool_y.tile([128, QB], F32, tag="psy",
                                                    name="psy", bufs=int(FLAGS["psy_bufs"]))
                    for kc in range(KC):
                        if (kc, qb) not in mts:
                            load_mt(kc, qb)
                        ps_s = pool_s.tile([128, 2 * QB], F32, tag="pss", name="ps_s")
                        for half in range(2):
                            po = half * DK
                            nc.tensor.matmul(ps_s[:, half * QB:(half + 1) * QB],
                                             kpt_sb[dc][po:po + DK, kc * 128:(kc + 1) * 128],
                                             qpt_sb[dc][po:po + DK, qb * QB:(qb + 1) * QB],
                                             start=True, stop=True)
                        if bg:
                            bg.pop(0)()
                        et = epool.tile([128, 2 * QB], CDT, tag="et", name="et",
                                        bufs=int(FLAGS["et_bufs"]))
                        nc.scalar.activation(out=et[:], in_=ps_s[:],
                                             func=mybir.ActivationFunctionType.Exp,
                                             scale=0.125)
                        ep = epool.tile([128, 2 * QB], CDT, tag="ep", name="ep",
                                        bufs=int(FLAGS["ep_bufs"]))
                        mtap = mts[(kc, qb)]
                        mb = bass.AP(tensor=mtap.tensor, offset=mtap.offset,
                                     ap=[list(mtap.ap[0]), [0, 2], [1, QB]])
                        gme = int(FLAGS["gps_mask_every"])
                        eng = nc.gpsimd if (gme and kc % gme == gme - 1) else nc.vector
                        eng.tensor_mul(ep.rearrange("p (a b) -> p a b", a=2),
                                       et.rearrange("p (a b) -> p a b", a=2), mb)
                        pending.append((qb, h0, kc, ep))
                        if len(pending) > int(FLAGS["lookahead"]):
                            emit_av(*pending.pop(0))
                    if dc == 3 and qb == 0:
                        while pending:
                            emit_av(*pending.pop(0))
                        _emit_wo(nc, t, w_sb, yt_sb, pool_y, opool, 0)
            while pending:
                emit_av(*pending.pop(0))
            _emit_wo(nc, t, w_sb, yt_sb, pool_y, opool, 1)
        else:
            for dc in range(4):
                emit_proj_qk(dc)
            for kc in range(KC):
                emit_proj_v(kc)
            for qb in range(NQ // QB):
                _emit_wo(nc, t, w_sb, yt_sb, pool_y, opool, qb)


def _emit_wo(nc, t, w_sb, yt_sb, pool_y, opool, qb):
    evac = nc.scalar.copy if qb == 1 else None
    for ec in range(4):
        ps = pool_y.tile([128, QB], F32, tag="psy", name="pso", bufs=int(FLAGS["psy_bufs"]))
        for dt_ in range(4):
            nc.tensor.matmul(ps[:], w_sb["wot"][dt_][:, ec * 128:(ec + 1) * 128],
                             yt_sb[dt_][:, qb * QB:(qb + 1) * QB],
                             start=(dt_ == 0), stop=(dt_ == 3))
        ot = opool.tile([128, QB], F32, tag="ot", name="ot")
        if evac is not None:
            evac(out=ot[:], in_=ps[:])
        else:
            nc.vector.tensor_copy(out=ot[:], in_=ps[:])
        nc.gpsimd.dma_start(out=t["out_t"].ap()[ec * 128:(ec + 1) * 128,
                                                qb * QB:(qb + 1) * QB],
                            in_=ot[:])


_NC_CACHE = {}


def build():
    if "nc" in _NC_CACHE:
        return _NC_CACHE["nc"], _NC_CACHE["t"]
    nc = bacc.Bacc(None, target_bir_lowering=False, debug=False)
    t = {
        "qt": nc.dram_tensor("qt", [D, NQ], CDT, kind="ExternalInput"),
        "kt": nc.dram_tensor("kt", [D, N], CDT, kind="ExternalInput"),
        "vt": nc.dram_tensor("vt", [D, N], CDT, kind="ExternalInput"),
        "mt": nc.dram_tensor("mt", [N, NQ], BF16, kind="ExternalInput"),
        "qmr": nc.dram_tensor("qmr", [1, NQ], BF16, kind="ExternalInput"),
        "kmr": nc.dram_tensor("kmr", [1, N], BF16, kind="ExternalInput"),
        "kmq": nc.dram_tensor("kmq", [1, NQ], BF16, kind="ExternalInput"),
        "wqt": nc.dram_tensor("wqt", [D, D], CDT, kind="ExternalInput"),
        "wkt": nc.dram_tensor("wkt", [D, D], CDT, kind="ExternalInput"),
        "wvt": nc.dram_tensor("wvt", [D, D], CDT, kind="ExternalInput"),
        "wot": nc.dram_tensor("wot", [D, D], CDT, kind="ExternalInput"),
        "out_t": nc.dram_tensor("out_t", [D, NQ], F32, kind="ExternalOutput"),
    }
    _emit(nc, t)
    nc.compile()
    _NC_CACHE["nc"] = nc
    _NC_CACHE["t"] = t
    return nc, t


def make_in_maps(Q, K, V, q_mas, k_mas, att_mas, WQ, WK, WV, WO):
    Q, K, V = (np.asarray(x, np.float32) for x in (Q, K, V))
    q_mas = np.asarray(q_mas, np.float32).reshape(BS, N)
    k_mas = np.asarray(k_mas, np.float32).reshape(BS, N)
    att_mas = np.asarray(att_mas, np.float32)
    wqt = np.ascontiguousarray(np.asarray(WQ, np.float32).T).astype(NP_CDT)
    wkt = np.ascontiguousarray(np.asarray(WK, np.float32).T).astype(NP_CDT)
    wvt = np.ascontiguousarray(np.asarray(WV, np.float32).T).astype(NP_CDT)
    wot = np.ascontiguousarray(np.asarray(WO, np.float32).T).astype(NP_CDT)
    in_maps = []
    for c in range(NCORES):
        bs, qh = c // 2, c % 2
        q0 = qh * NQ
        qt = np.ascontiguousarray(Q[bs].T[:, q0:q0 + NQ]).astype(NP_CDT)
        kt = np.ascontiguousarray(K[bs].T).astype(NP_CDT)
        vt = np.ascontiguousarray(V[bs].T).astype(NP_CDT)
        mt = np.ascontiguousarray(att_mas[bs].T[:, q0:q0 + NQ]).astype(ml_dtypes.bfloat16)
        in_maps.append({
            "qt": qt, "kt": kt, "vt": vt, "mt": mt,
            "qmr": q_mas[bs, q0:q0 + NQ].reshape(1, NQ).astype(ml_dtypes.bfloat16),
            "kmr": k_mas[bs].reshape(1, N).astype(ml_dtypes.bfloat16),
            "kmq": k_mas[bs, q0:q0 + NQ].reshape(1, NQ).astype(ml_dtypes.bfloat16),
            "wqt": wqt, "wkt": wkt, "wvt": wvt, "wot": wot,
        })
    return in_maps


def kernel(Q, K, V, q_mas, k_mas, att_mas, WQ, WK, WV, WO):
    nc, _ = build()
    in_maps = make_in_maps(Q, K, V, q_mas, k_mas, att_mas, WQ, WK, WV, WO)
    res = bass_utils.run_bass_kernel_spmd(nc, in_maps, core_ids=list(range(NCORES)))
    out = np.empty((BS, N, D), np.float32)
    for c in range(NCORES):
        bs, qh = c // 2, c % 2
        q0 = qh * NQ
        out[bs, q0:q0 + NQ, :] = res.results[c]["out_t"].T
    return out



# revision 64
# speedup vs baseline: 1.0021x; 1.0021x over previous
"""Bass/Tile multi-head-attention kernel for Trainium2, SPMD over 8 NeuronCores.

Sharding: core c = bs*2 + qhalf  (batch-parallel x query-half).  Each core
computes the full output rows for its (batch, 1024-query) slice; host glue
only slices / transposes / concatenates (no arithmetic on host).

Device math per core (bs, q0):
  QpT = (WQ^T)^T-contract (qm . Q)^T        [d, q]   (mask folded into PSUM evac)
  KpT = ... (km . K)^T                      [d, k]
  Vp  = (km . V) proj, heads interleaved    [k, 8, 65] with ones col at 64
  per head h, q-block qb (512):
    S^T[k, q] = KpT_h^T-slice . QpT_h       (PE, contraction d=64)
    E = exp(S^T / 8)                        (ACT, PSUM->SBUF)
    EP = E * mask^T                         (DVE, bf16 2x)
    [Y^T; rowsum] += [Vp_h | 1]^T . EP      (PE, accumulated over k)
    Y^T *= km(q) / rowsum                   (recip + partition_broadcast + DVE)
  out^T = WO^T-contract . Y^T               (PE)  -> DRAM [e, q] fp32
"""

import numpy as np
import ml_dtypes

import concourse.bass as bass
import concourse.bacc as bacc
import concourse.mybir as mybir
import concourse.tile as tile
from concourse import bass_utils

BS, N, D, H, DK = 4, 2048, 512, 8, 64
NCORES = 8
NQ = N // 2          # queries per core
QB = 512             # query block
KC = N // 128        # 16 key chunks of 128
F32 = mybir.dt.float32
BF16 = mybir.dt.bfloat16

# compute dtype for matmuls / staged activations: "bf16" or "f32"
COMPUTE = "bf16"
CDT = BF16 if COMPUTE == "bf16" else F32
NP_CDT = ml_dtypes.bfloat16 if COMPUTE == "bf16" else np.float32

# debug/bisection knobs (all default to production behavior)
FLAGS = {
    "bcast": "gpsimd",   # "gpsimd" | "dram"  — scale row broadcast mechanism
    "const_scale": False,  # replace recip+kmq by 1.0 (timing bisect only)
    "no_av": False,       # skip AV matmuls + evac (timing bisect only)
    "av_h0": False,       # all AV lhsT from head-0 slice (alignment probe)
    "no_exp": False,      # skip exp+mask (AV consumes garbage; timing only)
    "no_attn": False,     # skip whole attention loop (timing only)
    "reps": 1,            # emit the body N times (device-time measurement)
    "lookahead": 11,       # AV pipeline lookahead (groups)
    "proj_in_pss": False,  # proj psum shares the S-tile pool (frees a bank for psy)
    "psy_bufs": 3,
    "et_bufs": 6,
    "ep_bufs": 9,
    "gps_mask_every": 0,  # every Nth mask-mul runs on GPSIMD (0 = never)
    "warmup_mms": 20,     # dummy matmuls during the DMA ramp (HAM warm-up) +
                          # a dummy Exp to preload the ACT table set
}


def _emit(nc, t):
    """Emit the whole per-core program inside a TileContext."""
    with tile.TileContext(nc) as tc:
        for rep in range(int(FLAGS["reps"])):
            _emit_body(nc, tc, t, sfx=f"r{rep}" if rep else "")


def _emit_body(nc, tc, t, sfx=""):
    import contextlib
    ctx = contextlib.ExitStack()
    with ctx:
        persist = ctx.enter_context(tc.tile_pool(name="persist" + sfx, bufs=1))
        raw = ctx.enter_context(tc.tile_pool(name="raw" + sfx, bufs=1))

        # ---- small constants (loads emitted after the critical kt wave) ----
        qm_b = persist.tile([128, NQ], BF16, tag="qm_b")
        km_b = persist.tile([128, N], BF16, tag="km_b")
        km_sbr = persist.tile([128, KC], BF16, tag="km_sbr")
        km_sb = persist.tile([128, KC], F32, tag="km_sb")
        kmq_sb = persist.tile([1, NQ], BF16, tag="kmq_sb")

        def load_consts():
            ap = t["qmr"].ap()
            nc.sync.dma_start(out=qm_b[:], in_=bass.AP(tensor=ap.tensor, offset=ap.offset,
                                                       ap=[[0, 128], [1, NQ]]))
            ap = t["kmr"].ap()
            nc.sync.dma_start(out=km_b[:], in_=bass.AP(tensor=ap.tensor, offset=ap.offset,
                                                       ap=[[0, 128], [1, N]]))
            nc.sync.dma_start(out=km_sbr[:], in_=bass.AP(tensor=ap.tensor, offset=ap.offset,
                                                         ap=[[1, 128], [128, KC]]))
            nc.vector.tensor_copy(out=km_sb[:], in_=km_sbr[:])
            nc.sync.dma_start(out=kmq_sb[:], in_=t["kmq"].ap())

        # ---- weights (tiles created; loads emitted in criticality order) ----
        w_sb = {}
        for wname in ("wqt", "wkt", "wvt", "wot"):
            w_sb[wname] = [persist.tile([128, D], CDT, tag=f"{wname}{ct}", name=f"{wname}{ct}")
                           for ct in range(4)]

        def load_w(wname):
            for ct in range(4):
                nc.gpsimd.dma_start(out=w_sb[wname][ct][:],
                                    in_=t[wname].ap()[ct * 128:(ct + 1) * 128, :])

        # ---- persistent result tensors -------------------------------------
        qpt_sb = [persist.tile([128, NQ], CDT, tag=f"qpt{i}", name=f"qpt{i}") for i in range(4)]
        kpt_sb = [persist.tile([128, N], CDT, tag=f"kpt{i}", name=f"kpt{i}") for i in range(4)]
        v_sb = [persist.tile([128, H, DK + 1], CDT, tag=f"v{i}", name=f"v{i}") for i in range(KC)]
        yt_sb = [persist.tile([128, NQ], CDT, tag=f"yt{i}", name=f"yt{i}") for i in range(4)]
        if FLAGS["no_attn"] or FLAGS["no_av"]:
            for yt in yt_sb:
                nc.vector.memset(yt[:], 0.0)
        mt_ap = t["mt"].ap()

        # raw activations (alive for the whole kernel; proj is dc-interleaved)
        # loads split into column halves and emitted in criticality order:
        # qt+wqt (first proj) -> kt+wkt (first S-MM) -> vt+wvt -> wot
        qt_sb, kt_sb, vt_sb = [], [], []
        for nm, lst, cols in (("qt", qt_sb, NQ), ("kt", kt_sb, N), ("vt", vt_sb, N)):
            for ct in range(4):
                lst.append(raw.tile([128, cols], CDT, tag=f"{nm}{ct}", name=f"{nm}{ct}"))

        def load_raw(nm, lst, cols):
            for ct in range(4):
                for hlf in range(2):
                    c0 = hlf * (cols // 2)
                    nc.sync.dma_start(out=lst[ct][:, c0:c0 + cols // 2],
                                      in_=t[nm].ap()[ct * 128:(ct + 1) * 128, c0:c0 + cols // 2])

        mpool = ctx.enter_context(tc.tile_pool(name="mts" + sfx, bufs=1))
        mts = {}

        def load_mt(kc, qb):
            mtile = mpool.tile([128, QB], BF16, tag=f"mt{kc}_{qb}", name=f"mt{kc}_{qb}")
            off = (kc * 128) * NQ + qb * QB
            eng = nc.gpsimd if (kc + qb) % 2 else nc.sync
            eng.dma_start(out=mtile[:],
                          in_=bass.AP(tensor=mt_ap.tensor, offset=mt_ap.offset + off,
                                      ap=[[NQ, 128], [1, QB]]))
            mts[(kc, qb)] = mtile

        load_consts()
        load_raw("qt", qt_sb, NQ)
        load_w("wqt")
        load_raw("kt", kt_sb, N)
        load_w("wkt")
        load_raw("vt", vt_sb, N)
        load_w("wvt")
        for kc in range(4):
            load_mt(kc, 0)
        load_w("wot")

        pp = None if FLAGS["proj_in_pss"] else ctx.enter_context(tc.tile_pool(name="psproj" + sfx, bufs=1, space="PSUM"))

        def proj_ps():
            if FLAGS["proj_in_pss"]:
                return pool_s.tile([128, 2 * QB], F32, tag="pss", name="ps")[:, 0:QB]
            return pp.tile([128, QB], F32, tag="ps", name="ps")

        def emit_proj_q_blk(dc, blk):
            ps = proj_ps()
            for ct in range(4):
                nc.tensor.matmul(ps[:], w_sb["wqt"][ct][:, dc * 128:(dc + 1) * 128],
                                 qt_sb[ct][:, blk * QB:(blk + 1) * QB],
                                 start=(ct == 0), stop=(ct == 3))
            nc.vector.tensor_mul(qpt_sb[dc][:, blk * QB:(blk + 1) * QB],
                                 ps[:], qm_b[:, blk * QB:(blk + 1) * QB])

        def emit_proj_k_blk(dc, blk):
            ps = proj_ps()
            for ct in range(4):
                nc.tensor.matmul(ps[:], w_sb["wkt"][ct][:, dc * 128:(dc + 1) * 128],
                                 kt_sb[ct][:, blk * QB:(blk + 1) * QB],
                                 start=(ct == 0), stop=(ct == 3))
            nc.vector.tensor_mul(kpt_sb[dc][:, blk * QB:(blk + 1) * QB],
                                 ps[:], km_b[:, blk * QB:(blk + 1) * QB])

        def emit_proj_qk(dc):
            for blk in range(NQ // QB):
                emit_proj_q_blk(dc, blk)
            for blk in range(N // QB):
                emit_proj_k_blk(dc, blk)

        def emit_proj_v(kc):
            ps = proj_ps()
            for ct in range(4):
                nc.tensor.matmul(ps[:], vt_sb[ct][:, kc * 128:(kc + 1) * 128],
                                 w_sb["wvt"][ct][:], start=(ct == 0), stop=(ct == 3))
            nc.vector.tensor_scalar_mul(v_sb[kc][:, :, 0:DK],
                                        ps.rearrange("p (h e) -> p h e", h=H),
                                        km_sb[:, kc:kc + 1])
            nc.gpsimd.memset(v_sb[kc][:, :, DK:DK + 1], 1.0)

        # ---- attention (dc-interleaved with projections) --------------------
        pool_s = ctx.enter_context(tc.tile_pool(name="pss" + sfx, bufs=2, space="PSUM"))
        pool_y = ctx.enter_context(tc.tile_pool(name="psy" + sfx, bufs=2, space="PSUM"))
        epool = ctx.enter_context(tc.tile_pool(name="eps" + sfx, bufs=1))
        spool = ctx.enter_context(tc.tile_pool(name="smalls" + sfx, bufs=4))
        opool = ctx.enter_context(tc.tile_pool(name="osb" + sfx, bufs=3))

        psy = {}
        pending = []

        # PE/ACT warm-up during the input-DMA ramp: dummy matmuls (no DMA
        # deps) release the HAM clock gate, and a dummy Exp pulls the ACT
        # table load off the first real exp's critical path.
        nwarm = int(FLAGS["warmup_mms"])
        if nwarm:
            # borrow kpt_sb[0] pre-write (WAR: its proj evac waits on these)
            # and dump into yt_sb[0], which is fully overwritten by yn evacs.
            scratch = kpt_sb[0]
            nc.vector.memset(scratch[:, 0:QB], 0.0)
            ps_w = pp.tile([128, QB], F32, tag="ps", name="ps_w") if pp is not None \
                else pool_s.tile([128, 2 * QB], F32, tag="pss", name="ps_w")[:, 0:QB]
            for _ in range(nwarm):
                nc.tensor.matmul(ps_w[:], scratch[:, 0:128], scratch[:, 0:QB],
                                 start=True, stop=True)
            nc.scalar.activation(out=yt_sb[0][:, 0:QB], in_=ps_w[:],
                                 func=mybir.ActivationFunctionType.Exp, scale=0.125)


        if not FLAGS["no_attn"]:
            # Head-pair stream: per (dc, qb, kc) one psum tile holds
            # [S^T(h0,kc) | S^T(h1,kc)] (partition-offset 0/64 lhsT slices run
            # concurrently on disjoint PE row groups). One exp over both, two
            # mask muls sharing one mask tile, two AV accumulations.
            def V(kc):
                return lambda: emit_proj_v(kc)

            def Kb(dc, blk):
                return lambda: emit_proj_k_blk(dc, blk)

            def Qb(dc, blk):
                return lambda: emit_proj_q_blk(dc, blk)

            bg_by_dc = {
                0: [V(kc) for kc in range(6, KC)]
                   + [Qb(1, 0), Qb(1, 1), Kb(1, 0), Kb(1, 1), Kb(1, 2), Kb(1, 3)],
                1: [Qb(2, 0), Qb(2, 1), Kb(2, 0), Kb(2, 1), Kb(2, 2), Kb(2, 3)],
                2: [Qb(3, 0), Qb(3, 1), Kb(3, 0), Kb(3, 1), Kb(3, 2), Kb(3, 3)],
            }
            emit_proj_qk(0)
            for kc in range(6):
                emit_proj_v(kc)

            def emit_av(qb, h0, kc, ep):
                for half, h in ((0, h0), (1, h0 + 1)):
                    hh = 0 if FLAGS["av_h0"] else h
                    nc.tensor.matmul(psy[(h, qb)][0:DK + 1, :], v_sb[kc][:, hh, :],
                                     ep[:, half * QB:(half + 1) * QB],
                                     start=(kc == 0), stop=(kc == KC - 1))
                if kc == KC - 1:
                    for h in (h0, h0 + 1):
                        ps_y = psy[(h, qb)]
                        scl = spool.tile([1, QB], F32, tag="scl", name="scl")
                        if FLAGS["const_scale"]:
                            nc.vector.memset(scl[:], 1.0)
                        else:
                            rec = spool.tile([1, QB], F32, tag="rec", name="rec")
                            nc.vector.reciprocal(rec[:], ps_y[DK:DK + 1, :])
                            nc.vector.tensor_mul(scl[:], rec[:],
                                                 kmq_sb[:, qb * QB:(qb + 1) * QB])
                        sclb = spool.tile([DK, QB], F32, tag="sclb", name="sclb")
                        nc.gpsimd.partition_broadcast(sclb[:], scl[:])
                        po = (h % 2) * DK
                        nc.vector.tensor_mul(yt_sb[h // 2][po:po + DK,
                                                           qb * QB:(qb + 1) * QB],
                                             ps_y[0:DK, :], sclb[:])

            for dc in range(4):
                h0 = 2 * dc
                bg = bg_by_dc.get(dc, [])
                for qb in range(NQ // QB):
                    psy[(h0, qb)] = pool_y.tile([128, QB], F32, tag="psy",
                                                name="psy", bufs=int(FLAGS["psy_bufs"]))
                    psy[(h0 + 1, qb)] = pool_y.tile([128, QB], F32, tag="psy",
                                                    name="psy", bufs=int(FLAGS["psy_bufs"]))
                    for kc in range(KC):
                        if (kc, qb) not in mts:
                            load_mt(kc, qb)
                        ps_s = pool_s.tile([128, 2 * QB], F32, tag="pss", name="ps_s")
                        for half in range(2):
                            po = half * DK
                            nc.tensor.matmul(ps_s[:, half * QB:(half + 1) * QB],
                                             kpt_sb[dc][po:po + DK, kc * 128:(kc + 1) * 128],
                                             qpt_sb[dc][po:po + DK, qb * QB:(qb + 1) * QB],
                                             start=True, stop=True)
                        if bg:
                            bg.pop(0)()
                        et = epool.tile([128, 2 * QB], CDT, tag="et", name="et",
                                        bufs=int(FLAGS["et_bufs"]))
                        nc.scalar.activation(out=et[:], in_=ps_s[:],
                                             func=mybir.ActivationFunctionType.Exp,
                                             scale=0.125)
                        ep = epool.tile([128, 2 * QB], CDT, tag="ep", name="ep",
                                        bufs=int(FLAGS["ep_bufs"]))
                        mtap = mts[(kc, qb)]
                        mb = bass.AP(tensor=mtap.tensor, offset=mtap.offset,
                                     ap=[list(mtap.ap[0]), [0, 2], [1, QB]])
                        gme = int(FLAGS["gps_mask_every"])
                        eng = nc.gpsimd if (gme and kc % gme == gme - 1) else nc.vector
                        eng.tensor_mul(ep.rearrange("p (a b) -> p a b", a=2),
                                       et.rearrange("p (a b) -> p a b", a=2), mb)
                        pending.append((qb, h0, kc, ep))
                        if len(pending) > int(FLAGS["lookahead"]):
                            emit_av(*pending.pop(0))
                    if dc == 3 and qb == 0:
                        while pending:
                            emit_av(*pending.pop(0))
                        _emit_wo(nc, t, w_sb, yt_sb, pool_y, opool, 0)
            while pending:
                emit_av(*pending.pop(0))
            _emit_wo(nc, t, w_sb, yt_sb, pool_y, opool, 1)
        else:
            for dc in range(4):
                emit_proj_qk(dc)
            for kc in range(KC):
                emit_proj_v(kc)
            for qb in range(NQ // QB):
                _emit_wo(nc, t, w_sb, yt_sb, pool_y, opool, qb)


def _emit_wo(nc, t, w_sb, yt_sb, pool_y, opool, qb):
    evac = nc.scalar.copy if qb == 1 else None
    for ec in range(4):
        ps = pool_y.tile([128, QB], F32, tag="psy", name="pso", bufs=int(FLAGS["psy_bufs"]))
        for dt_ in range(4):
            nc.tensor.matmul(ps[:], w_sb["wot"][dt_][:, ec * 128:(ec + 1) * 128],
                             yt_sb[dt_][:, qb * QB:(qb + 1) * QB],
                             start=(dt_ == 0), stop=(dt_ == 3))
        ot = opool.tile([128, QB], F32, tag="ot", name="ot")
        if evac is not None:
            evac(out=ot[:], in_=ps[:])
        else:
            nc.vector.tensor_copy(out=ot[:], in_=ps[:])
        nc.gpsimd.dma_start(out=t["out_t"].ap()[ec * 128:(ec + 1) * 128,
                                                qb * QB:(qb + 1) * QB],
                            in_=ot[:])


_NC_CACHE = {}


def build():
    if "nc" in _NC_CACHE:
        return _NC_CACHE["nc"], _NC_CACHE["t"]
    nc = bacc.Bacc(None, target_bir_lowering=False, debug=False)
    t = {
        "qt": nc.dram_tensor("qt", [D, NQ], CDT, kind="ExternalInput"),
        "kt": nc.dram_tensor("kt", [D, N], CDT, kind="ExternalInput"),
        "vt": nc.dram_tensor("vt", [D, N], CDT, kind="ExternalInput"),
        "mt": nc.dram_tensor("mt", [N, NQ], BF16, kind="ExternalInput"),
        "qmr": nc.dram_tensor("qmr", [1, NQ], BF16, kind="ExternalInput"),
        "kmr": nc.dram_tensor("kmr", [1, N], BF16, kind="ExternalInput"),
        "kmq": nc.dram_tensor("kmq", [1, NQ], BF16, kind="ExternalInput"),
        "wqt": nc.dram_tensor("wqt", [D, D], CDT, kind="ExternalInput"),
        "wkt": nc.dram_tensor("wkt", [D, D], CDT, kind="ExternalInput"),
        "wvt": nc.dram_tensor("wvt", [D, D], CDT, kind="ExternalInput"),
        "wot": nc.dram_tensor("wot", [D, D], CDT, kind="ExternalInput"),
        "out_t": nc.dram_tensor("out_t", [D, NQ], F32, kind="ExternalOutput"),
    }
    _emit(nc, t)
    nc.compile()
    _NC_CACHE["nc"] = nc
    _NC_CACHE["t"] = t
    return nc, t


def make_in_maps(Q, K, V, q_mas, k_mas, att_mas, WQ, WK, WV, WO):
    Q, K, V = (np.asarray(x, np.float32) for x in (Q, K, V))
    q_mas = np.asarray(q_mas, np.float32).reshape(BS, N)
    k_mas = np.asarray(k_mas, np.float32).reshape(BS, N)
    att_mas = np.asarray(att_mas, np.float32)
    wqt = np.ascontiguousarray(np.asarray(WQ, np.float32).T).astype(NP_CDT)
    wkt = np.ascontiguousarray(np.asarray(WK, np.float32).T).astype(NP_CDT)
    wvt = np.ascontiguousarray(np.asarray(WV, np.float32).T).astype(NP_CDT)
    wot = np.ascontiguousarray(np.asarray(WO, np.float32).T).astype(NP_CDT)
    in_maps = []
    for c in range(NCORES):
        bs, qh = c // 2, c % 2
        q0 = qh * NQ
        qt = np.ascontiguousarray(Q[bs].T[:, q0:q0 + NQ]).astype(NP_CDT)
        kt = np.ascontiguousarray(K[bs].T).astype(NP_CDT)
        vt = np.ascontiguousarray(V[bs].T).astype(NP_CDT)
        mt = np.ascontiguousarray(att_mas[bs].T[:, q0:q0 + NQ]).astype(ml_dtypes.bfloat16)
        in_maps.append({
            "qt": qt, "kt": kt, "vt": vt, "mt": mt,
            "qmr": q_mas[bs, q0:q0 + NQ].reshape(1, NQ).astype(ml_dtypes.bfloat16),
            "kmr": k_mas[bs].reshape(1, N).astype(ml_dtypes.bfloat16),
            "kmq": k_mas[bs, q0:q0 + NQ].reshape(1, NQ).astype(ml_dtypes.bfloat16),
            "wqt": wqt, "wkt": wkt, "wvt": wvt, "wot": wot,
        })
    return in_maps


def kernel(Q, K, V, q_mas, k_mas, att_mas, WQ, WK, WV, WO):
    nc, _ = build()
    in_maps = make_in_maps(Q, K, V, q_mas, k_mas, att_mas, WQ, WK, WV, WO)
    res = bass_utils.run_bass_kernel_spmd(nc, in_maps, core_ids=list(range(NCORES)))
    out = np.empty((BS, N, D), np.float32)
    for c in range(NCORES):
        bs, qh = c // 2, c % 2
        q0 = qh * NQ
        out[bs, q0:q0 + NQ, :] = res.results[c]["out_t"].T
    return out

